# revision 6
# baseline (speedup 1.0000x reference)
"""Trainium2 Bass kernel for the AxialShift block (4x96x256x256, fp32).

Self-contained: builds an 8-core SPMD Bass program (one NeuronCore per
half-sample), compiles it once, and runs it via run_bass_kernel_spmd.

Sharding: core k handles sample k//2, horizontal half k%2 (128 rows).
Per-core pipeline (all resident in SBUF, x read once, out written once):
  phase A : conv1 (96x96 matmul, fp32r) over the core's 130-row frame
            (128 true rows + 1 halo row on each side); result stored as
            fp16 into a zero-padded [96, 130, 258] activation buffer;
            per-channel bn_stats partials on the true region.
  AR1     : 8-byte AllReduce of GroupNorm-1 partial sums over the
            2-core group that shares each sample.
  GN-apply: fused per-channel scale/bias + erf-Gelu in place.
  phase B : the 4 axial-shift branch convs as 12 K=32 chunk matmuls per
            512-pixel tile (shifts become free-dim offsets into the
            padded buffer; PE row-tiling runs the 3 chunks of a branch
            concurrently), fused bias+Gelu on ScalarE, branch sum on
            VectorE, bn_stats partials for GroupNorm-2.
  AR2     : second 8-byte AllReduce.
  phase C : GroupNorm-2 folded into conv3 (weights scaled by gamma2/std
            on device, mean/bias folded into a per-channel bias applied
            during the PSUM->SBUF evacuation), DMA out.
"""
import sys

sys.path.insert(0, "/opt/trn_rl_repo")

import numpy as np

import concourse.bass as bass
import concourse.bacc as bacc
import concourse.tile as tile
from concourse import mybir

F32 = mybir.dt.float32
F32R = mybir.dt.float32r
F16 = mybir.dt.float16

C = 96
H = 256
W = 256
B = 4
WP = W + 2
N_CORES = 8
ROWS_HALF = H // 2
GROUPS = [[0, 1], [2, 3], [4, 5], [6, 7]]
EPS = 1e-5
AF = mybir.ActivationFunctionType

# (dh, dw) read offsets per chunk j=0,1,2 (s_j = -1, 0, +1) for the four
# branches: value(out_pixel p) = xact[p + (dh, dw)].
#   x_lr    : roll(+s) along W  -> dw = -s
#   x_l_diag: x_lr then roll(+s) along H -> (dh, dw) = (-s, -s)
#   x_td    : roll(+s) along H  -> dh = -s
#   x_r_diag: x_td then roll(-s) along W -> (dh, dw) = (-s, +s)
BR_LR = [(0, 1), (0, 0), (0, -1)]
BR_LDIAG = [(1, 1), (0, 0), (-1, -1)]
BR_TD = [(1, 0), (0, 0), (-1, 0)]
BR_RDIAG = [(1, -1), (0, 0), (-1, 1)]


def _bcast(ap, nparts):
    """Broadcast a [1, k] AP across nparts partitions (stride-0 partition dim)."""
    return bass.AP(tensor=ap.tensor, offset=ap.offset,
                   ap=[[0, nparts]] + list(ap.ap[1:]))


def _emit(nc, tc, ctx, RH, groups, io):
    RF = RH + 2           # frame rows incl. halo
    NT = RH // 2          # 2-row (512-px) output tiles
    NPT = RF // 2         # 2-row pixel tiles covering the frame
    inv_n = 1.0 / (2 * C)  # stats partials: sum over 2 cores x 96 channels

    consts = ctx.enter_context(tc.tile_pool(name="consts", bufs=1))
    big = ctx.enter_context(tc.tile_pool(name="big", bufs=1))
    xin = ctx.enter_context(tc.tile_pool(name="xin", bufs=3))
    psum = ctx.enter_context(tc.tile_pool(name="psum", bufs=2, space="PSUM"))
    gst = ctx.enter_context(tc.tile_pool(name="gst", bufs=3))
    tmp = ctx.enter_context(tc.tile_pool(name="tmp", bufs=4))
    ost = ctx.enter_context(tc.tile_pool(name="ost", bufs=2))
    dram = ctx.enter_context(tc.tile_pool(name="dram", bufs=1, space="DRAM"))

    # ---------------- resident buffers & constants ----------------
    xact = big.tile([C, RF, WP], F16)        # padded activation frame
    opre = big.tile([C, RH * W], F16)        # branch-sum (pre-GN2), flat

    w1t = consts.tile([C, C], F32)
    nc.sync.dma_start(out=w1t[:], in_=io["w1t"][:])
    wbm = consts.tile([C, 6 * C], F16)   # 6 chunk-masked lhsT blocks
    nc.sync.dma_start(out=wbm[:], in_=io["wbm"][:])
    w3gt = consts.tile([C, C], F32)
    nc.sync.dma_start(out=w3gt[:], in_=io["w3gt"][:])
    cols = consts.tile([C, 7], F32)          # b1,g1,beta1,b21,b22,w3base,w3v
    nc.sync.dma_start(out=cols[:], in_=io["cols"][:])
    em = consts.tile([C, 2], F32)            # halo-row validity mask
    nc.gpsimd.dma_start(out=em[:], in_=_bcast(io["em"][:], C))
    ones96 = consts.tile([C, 1], F32)
    nc.vector.memset(ones96[:], 1.0)
    epsb = consts.tile([C, 1], F32)
    nc.vector.memset(epsb[:], EPS)

    # zero the width-pad columns (kept zero through phase A; re-zeroed
    # after the in-place Gelu pollutes them)
    nc.vector.memset(xact[:, :, 0:1], 0.0)
    nc.vector.memset(xact[:, :, WP - 1:WP], 0.0)

    stats1 = consts.tile([C, NPT, 6], F32)
    stats2 = consts.tile([C, NT, 6], F32)

    # ---------------- phase A: conv1 + GN1 partials ----------------
    for b0 in range(0, NPT, 4):
        nb = min(4, NPT - b0)
        xt = xin.tile([C, 8, W], F32, tag="xt")
        nc.sync.dma_start(out=xt[:, 0:2 * nb, :],
                          in_=io["xs"][:, 2 * b0:2 * b0 + 2 * nb, :])
        pt = psum.tile([C, 4, 512], F32, tag="pt")
        for j in range(nb):
            nc.tensor.matmul(out=pt[:, j, :],
                             lhsT=w1t[:],
                             rhs=xt[:, 2 * j:2 * j + 2, :],
                             start=True, stop=True)
        # PSUM -> padded fp16 frame (cast); [C, nb, 2, 256] view both sides
        nc.scalar.copy(
            out=xact[:, 2 * b0:2 * b0 + 2 * nb, 1:W + 1].rearrange(
                "p (n r) w -> p n r w", r=2),
            in_=pt[:, 0:nb, :].rearrange("p n (r w) -> p n r w", w=W))
        for j in range(nb):
            ti = b0 + j
            if ti == 0:
                src = pt[:, j, 256:512]        # row 0 is halo
            elif ti == NPT - 1:
                src = pt[:, j, 0:256]          # last row is halo
            else:
                src = pt[:, j, :]
            nc.vector.bn_stats(out=stats1[:, ti, :], in_=src)

    # ---- GN1 partial reduction + AllReduce
    mv1 = consts.tile([C, 2], F32)
    nc.vector.bn_aggr(out=mv1[:], in_=stats1[:])
    pack1 = consts.tile([C, 2], F32)
    nc.vector.tensor_add(out=pack1[:, 0:1], in0=mv1[:, 0:1], in1=cols[:, 0:1])
    t1sq = consts.tile([C, 1], F32)
    nc.vector.tensor_mul(out=t1sq[:], in0=pack1[:, 0:1], in1=pack1[:, 0:1])
    nc.vector.tensor_add(out=pack1[:, 1:2], in0=mv1[:, 1:2], in1=t1sq[:])
    spt = psum.tile([C, 4, 512], F32, tag="pt")
    nc.tensor.matmul(out=spt[0:1, 0, 0:2], lhsT=ones96[:], rhs=pack1[:],
                     start=True, stop=True)
    ar1_in = consts.tile([1, 2], F32)
    nc.scalar.copy(out=ar1_in[:], in_=spt[0:1, 0, 0:2])
    d1i = dram.tile([1, 2], F32)
    d1o = dram.tile([1, 2], F32)
    nc.sync.dma_start(out=d1i[:], in_=ar1_in[:])
    nc.gpsimd.collective_compute("AllReduce", mybir.AluOpType.add,
                                 replica_groups=groups,
                                 ins=[d1i.opt()], outs=[d1o.opt()])
    ar1 = consts.tile([C, 2], F32)
    nc.gpsimd.dma_start(out=ar1[:], in_=_bcast(d1o[:], C))

    # ---- GN1 scalars (computed redundantly on all 96 partitions)
    mu1 = consts.tile([C, 1], F32)
    nc.vector.tensor_scalar_mul(out=mu1[:], in0=ar1[:, 0:1], scalar1=inv_n)
    var1 = consts.tile([C, 1], F32)
    nc.vector.tensor_scalar_mul(out=var1[:], in0=ar1[:, 1:2], scalar1=inv_n)
    mu1sq = consts.tile([C, 1], F32)
    nc.vector.tensor_mul(out=mu1sq[:], in0=mu1[:], in1=mu1[:])
    nc.vector.tensor_sub(out=var1[:], in0=var1[:], in1=mu1sq[:])
    std1 = consts.tile([C, 1], F32)
    nc.scalar.activation(out=std1[:], in_=var1[:], func=AF.Sqrt, bias=epsb[:])
    inv1 = consts.tile([C, 1], F32)
    nc.vector.reciprocal(out=inv1[:], in_=std1[:])
    scale1 = consts.tile([C, 1], F32)
    nc.vector.tensor_mul(out=scale1[:], in0=inv1[:], in1=cols[:, 1:2])
    bias1 = consts.tile([C, 1], F32)
    nc.vector.tensor_sub(out=bias1[:], in0=cols[:, 0:1], in1=mu1[:])
    nc.vector.tensor_mul(out=bias1[:], in0=bias1[:], in1=scale1[:])
    nc.vector.tensor_add(out=bias1[:], in0=bias1[:], in1=cols[:, 2:3])

    # ---- GN1 apply + Gelu, in place over the padded frame
    rchunk = max(2, (RF + 9) // 10)
    r = 0
    while r < RF:
        rr = min(rchunk, RF - r)
        nc.scalar.activation(out=xact[:, r:r + rr, :], in_=xact[:, r:r + rr, :],
                             func=AF.Gelu, bias=bias1[:], scale=scale1[:])
        r += rr
    # sample-edge halo rows -> 0; re-zero the Gelu-polluted pad columns
    nc.vector.tensor_scalar_mul(out=xact[:, 0:1, :], in0=xact[:, 0:1, :],
                                scalar1=em[:, 0:1])
    nc.vector.tensor_scalar_mul(out=xact[:, RF - 1:RF, :],
                                in0=xact[:, RF - 1:RF, :], scalar1=em[:, 1:2])
    nc.vector.memset(xact[:, :, 0:1], 0.0)
    nc.vector.memset(xact[:, :, WP - 1:WP], 0.0)

    # ---------------- phase B: branch convs + Gelu + sum ----------------
    branches = [(0, BR_LR), (0, BR_LDIAG), (3, BR_TD), (3, BR_RDIAG)]
    for t in range(NT):
        pr = 2 * t + 1    # first padded row feeding this output tile
        pt = psum.tile([C, 4, 512], F32, tag="pt")
        for b, (wsel, ds) in enumerate(branches):
            for j, (dh, dw) in enumerate(ds):
                bi = wsel + j
                nc.tensor.matmul(
                    out=pt[:, b, :],
                    lhsT=wbm[:, bi * C:(bi + 1) * C],
                    rhs=xact[0:C,
                             pr + dh:pr + dh + 2,
                             1 + dw:1 + dw + W],
                    start=(j == 0), stop=(j == 2))
        g = gst.tile([C, 4, 512], F16, tag="g")
        nc.scalar.activation(out=g[:, 0:2, :], in_=pt[:, 0:2, :],
                             func=AF.Gelu, bias=cols[:, 3:4])
        nc.scalar.activation(out=g[:, 2:4, :], in_=pt[:, 2:4, :],
                             func=AF.Gelu, bias=cols[:, 4:5])
        o1 = tmp.tile([C, 512], F16, tag="o1")
        o2 = tmp.tile([C, 512], F16, tag="o2")
        nc.vector.tensor_add(out=o1[:], in0=g[:, 0, :], in1=g[:, 1, :])
        nc.vector.tensor_add(out=o2[:], in0=g[:, 2, :], in1=g[:, 3, :])
        od = opre[:, 512 * t:512 * (t + 1)]
        nc.vector.tensor_add(out=od, in0=o1[:], in1=o2[:])
        nc.vector.bn_stats(out=stats2[:, t, :], in_=od)

    # ---- GN2 partial reduction + AllReduce
    mv2 = consts.tile([C, 2], F32)
    nc.vector.bn_aggr(out=mv2[:], in_=stats2[:])
    pack2 = consts.tile([C, 2], F32)
    nc.vector.tensor_copy(out=pack2[:, 0:1], in_=mv2[:, 0:1])
    t2sq = consts.tile([C, 1], F32)
    nc.vector.tensor_mul(out=t2sq[:], in0=mv2[:, 0:1], in1=mv2[:, 0:1])
    nc.vector.tensor_add(out=pack2[:, 1:2], in0=mv2[:, 1:2], in1=t2sq[:])
    spt2 = psum.tile([C, 4, 512], F32, tag="pt")
    nc.tensor.matmul(out=spt2[0:1, 0, 0:2], lhsT=ones96[:], rhs=pack2[:],
                     start=True, stop=True)
    ar2_in = consts.tile([1, 2], F32)
    nc.scalar.copy(out=ar2_in[:], in_=spt2[0:1, 0, 0:2])
    d2i = dram.tile([1, 2], F32)
    d2o = dram.tile([1, 2], F32)
    nc.sync.dma_start(out=d2i[:], in_=ar2_in[:])
    nc.gpsimd.collective_compute("AllReduce", mybir.AluOpType.add,
                                 replica_groups=groups,
                                 ins=[d2i.opt()], outs=[d2o.opt()])
    ar2 = consts.tile([C, 2], F32)
    nc.gpsimd.dma_start(out=ar2[:], in_=_bcast(d2o[:], C))

    # ---- GN2 scalars; fold gamma2/std into conv3 weights, mean into bias
    mu2 = consts.tile([C, 1], F32)
    nc.vector.tensor_scalar_mul(out=mu2[:], in0=ar2[:, 0:1], scalar1=inv_n)
    var2 = consts.tile([C, 1], F32)
    nc.vector.tensor_scalar_mul(out=var2[:], in0=ar2[:, 1:2], scalar1=inv_n)
    mu2sq = consts.tile([C, 1], F32)
    nc.vector.tensor_mul(out=mu2sq[:], in0=mu2[:], in1=mu2[:])
    nc.vector.tensor_sub(out=var2[:], in0=var2[:], in1=mu2sq[:])
    std2 = consts.tile([C, 1], F32)
    nc.scalar.activation(out=std2[:], in_=var2[:], func=AF.Sqrt, bias=epsb[:])
    inv2 = consts.tile([C, 1], F32)
    nc.vector.reciprocal(out=inv2[:], in_=std2[:])
    w3ts = consts.tile([C, C], F16)
    nc.vector.tensor_scalar_mul(out=w3ts[:], in0=w3gt[:], scalar1=inv2[:])
    s2 = consts.tile([C, 1], F32)
    nc.vector.tensor_mul(out=s2[:], in0=inv2[:], in1=mu2[:])
    ccol = consts.tile([C, 1], F32)
    nc.vector.tensor_mul(out=ccol[:], in0=s2[:], in1=cols[:, 6:7])
    nc.vector.tensor_sub(out=ccol[:], in0=cols[:, 5:6], in1=ccol[:])

    # ---------------- phase C: conv3 + bias, DMA out ----------------
    for b0 in range(0, NT, 4):
        nb = min(4, NT - b0)
        pc = psum.tile([C, 4, 512], F32, tag="pt")
        for j in range(nb):
            tt = b0 + j
            nc.tensor.matmul(out=pc[:, j, :], lhsT=w3ts[:],
                             rhs=opre[:, 512 * tt:512 * (tt + 1)],
                             start=True, stop=True)
        o = ost.tile([C, 4, 512], F32, tag="o")
        nc.scalar.activation(out=o[:, 0:nb, :], in_=pc[:, 0:nb, :],
                             func=AF.Identity, bias=ccol[:])
        nc.sync.dma_start(
            out=io["out"][:, 2 * b0:2 * b0 + 2 * nb, :].rearrange(
                "p (n r) w -> p n r w", r=2),
            in_=o[:, 0:nb, :].rearrange("p n (r w) -> p n r w", w=W))


def build_program(rows_half=ROWS_HALF, n_cores=N_CORES, groups=None):
    """Build + bacc-compile the SPMD program. Returns (nc, io_names)."""
    import contextlib
    if groups is None:
        groups = [[i, i + 1] for i in range(0, n_cores, 2)]
    RF = rows_half + 2
    nc = bacc.Bacc("TRN2", target_bir_lowering=False, debug=False,
                   enable_asserts=False, num_devices=n_cores)
    io = {
        "xs": nc.dram_tensor("xs", [C, RF, W], F32, kind="ExternalInput").ap(),
        "em": nc.dram_tensor("em", [1, 2], F32, kind="ExternalInput").ap(),
        "w1t": nc.dram_tensor("w1t", [C, C], F32, kind="ExternalInput").ap(),
        "wbm": nc.dram_tensor("wbm", [C, 6 * C], F16, kind="ExternalInput").ap(),
        "w3gt": nc.dram_tensor("w3gt", [C, C], F32, kind="ExternalInput").ap(),
        "cols": nc.dram_tensor("cols", [C, 7], F32, kind="ExternalInput").ap(),
        "out": nc.dram_tensor("out", [C, rows_half, W], F32,
                              kind="ExternalOutput").ap(),
    }
    with tile.TileContext(nc) as tc:
        with contextlib.ExitStack() as ctx:
            _emit(nc, tc, ctx, rows_half, groups, io)
    nc.compile()
    return nc


def host_inputs(x, w1, b1, w21, b21, w22, b22, w3, b3,
                gn1_w, gn1_b, gn2_w, gn2_b, rows_half=ROWS_HALF):
    """Build the per-core in_maps for run_bass_kernel_spmd."""
    x = np.asarray(x, np.float32)
    nb, _, hh, _ = x.shape
    n_cores = nb * (hh // rows_half)
    w1 = np.asarray(w1, np.float32)
    w21 = np.asarray(w21, np.float32)
    w22 = np.asarray(w22, np.float32)
    w3 = np.asarray(w3, np.float32)
    wbm = np.zeros((C, 6 * C), np.float16)
    for wi, wmat in enumerate((w21, w22)):
        wt = np.ascontiguousarray(wmat.T).astype(np.float16)   # [k, m]
        for j in range(3):
            blk = np.zeros((C, C), np.float16)
            blk[32 * j:32 * j + 32, :] = wt[32 * j:32 * j + 32, :]
            wbm[:, (3 * wi + j) * C:(3 * wi + j + 1) * C] = blk
    shared = {
        "w1t": np.ascontiguousarray(w1.T),
        "wbm": wbm,
        "w3gt": np.ascontiguousarray((w3 * np.asarray(gn2_w)[None, :]).T),
        "cols": np.ascontiguousarray(np.stack(
            [np.asarray(b1, np.float32), np.asarray(gn1_w, np.float32),
             np.asarray(gn1_b, np.float32), np.asarray(b21, np.float32),
             np.asarray(b22, np.float32),
             (np.asarray(b3) + w3 @ np.asarray(gn2_b)).astype(np.float32),
             (w3 * np.asarray(gn2_w)[None, :]).sum(1).astype(np.float32)],
            axis=1)),
    }
    in_maps = []
    halves = hh // rows_half
    for k in range(n_cores):
        bidx, half = k // halves, k % halves
        h0 = half * rows_half
        slab = np.zeros((C, rows_half + 2, W), np.float32)
        lo, hi = h0 - 1, h0 + rows_half + 1
        slo, shi = max(lo, 0), min(hi, hh)
        slab[:, slo - lo:slo - lo + (shi - slo), :] = x[bidx, :, slo:shi, :]
        em = np.array([[1.0 if lo >= 0 else 0.0,
                        1.0 if hi <= hh else 0.0]], np.float32)
        in_maps.append({"xs": slab, "em": em, **shared})
    return in_maps


_PROGRAM = None


def kernel(x, w1, b1, w21, b21, w22, b22, w3, b3, gn1_w, gn1_b, gn2_w, gn2_b):
    global _PROGRAM
    from concourse.bass_utils import run_bass_kernel_spmd
    from concourse.bass_interp import get_hw_module
    if _PROGRAM is None:
        nc = build_program()
        nc.m = get_hw_module(nc.m)
        _PROGRAM = nc
    nc = _PROGRAM
    in_maps = host_inputs(x, w1, b1, w21, b21, w22, b22, w3, b3,
                          gn1_w, gn1_b, gn2_w, gn2_b)
    res = run_bass_kernel_spmd(nc, in_maps, core_ids=list(range(N_CORES)))
    out = np.empty((B, C, H, W), np.float32)
    for k in range(N_CORES):
        bidx, half = k // 2, k % 2
        out[bidx, :, half * ROWS_HALF:(half + 1) * ROWS_HALF, :] = \
            res.results[k]["out"]
    return out


# revision 7
# speedup vs baseline: 1.2248x; 1.2248x over previous
"""Trainium2 Bass kernel for the AxialShift block (4x96x256x256, fp32).

Self-contained: builds an 8-core SPMD Bass program (one NeuronCore per
half-sample), compiles it once, and runs it via run_bass_kernel_spmd.

Sharding: core k handles sample k//2, horizontal half k%2 (128 rows).
Per-core pipeline (x read once as fp16, out written once):
  phase A : conv1 (fp16 matmul, M padded to 128 for fast weight load)
            over the 130-row frame (128 true rows + 1 halo row each
            side); stored fp16 into a zero-padded [96, 130, 258] frame;
            per-channel bn_stats partials on the true region.
  AR1     : 8-byte AllReduce of GroupNorm-1 partials over the 2-core
            group sharing each sample.
  GN-apply: fused per-channel scale/bias + erf-Gelu in place (strided,
            pad columns stay zero).
  phase B : the 4 axial-shift branch convs as 12 chunk-masked K=96
            matmuls per 512-pixel tile (shifts are free-dim offsets
            into the padded frame), fused bias+Gelu on ScalarE, branch
            sum split across VectorE/GpSimd, bn_stats partials for GN2.
  AR2     : second 8-byte AllReduce.
  phase C : GroupNorm-2 folded into conv3 (weights scaled by gamma2/std
            on device; mean/bias folded into a per-channel bias applied
            by the VectorE PSUM->SBUF evacuation), DMA out.
"""
import sys

sys.path.insert(0, "/opt/trn_rl_repo")

import numpy as np

import concourse.bass as bass
import concourse.bacc as bacc
import concourse.tile as tile
from concourse import mybir

F32 = mybir.dt.float32
F16 = mybir.dt.float16

C = 96
M = 128           # matmul output width (96 channels + 32 zero pad, FWL)
H = 256
W = 256
B = 4
WP = W + 2
N_CORES = 8
ROWS_HALF = H // 2
EPS = 1e-5
AF = mybir.ActivationFunctionType
ALU = mybir.AluOpType

# (dh, dw) read offsets per chunk j=0,1,2 (s_j = -1, 0, +1):
BR_LR = [(0, 1), (0, 0), (0, -1)]
BR_LDIAG = [(1, 1), (0, 0), (-1, -1)]
BR_TD = [(1, 0), (0, 0), (-1, 0)]
BR_RDIAG = [(1, -1), (0, 0), (-1, 1)]


def _bcast(ap, nparts):
    return bass.AP(tensor=ap.tensor, offset=ap.offset,
                   ap=[[0, nparts]] + list(ap.ap[1:]))


def _emit(nc, tc, ctx, RH, groups, io):
    RF = RH + 2
    NT = RH // 2
    NPT = RF // 2
    inv_n = 1.0 / (len(groups[0]) * C)

    consts = ctx.enter_context(tc.tile_pool(name="consts", bufs=1))
    big = ctx.enter_context(tc.tile_pool(name="big", bufs=1))
    xin = ctx.enter_context(tc.tile_pool(name="xin", bufs=3))
    gst = ctx.enter_context(tc.tile_pool(name="gst", bufs=3))
    tmp = ctx.enter_context(tc.tile_pool(name="tmp", bufs=4))
    ost = ctx.enter_context(tc.tile_pool(name="ost", bufs=4))
    dram = ctx.enter_context(tc.tile_pool(name="dram", bufs=1, space="DRAM"))

    # ---------------- resident buffers & constants ----------------
    xact = big.tile([C, RF, WP], F16)
    opre = big.tile([C, RH * W], F16)

    w1t = consts.tile([C, M], F16)
    nc.sync.dma_start(out=w1t[:], in_=io["w1t"][:])
    wbm = consts.tile([C, 6 * M], F16)
    nc.sync.dma_start(out=wbm[:], in_=io["wbm"][:])
    w3gt = consts.tile([C, M], F32)
    nc.sync.dma_start(out=w3gt[:], in_=io["w3gt"][:])
    cols = consts.tile([C, 7], F32)
    nc.sync.dma_start(out=cols[:], in_=io["cols"][:])
    em = consts.tile([C, 2], F32)
    nc.gpsimd.dma_start(out=em[:], in_=_bcast(io["em"][:], C))
    ones96 = consts.tile([C, 1], F32)
    nc.vector.memset(ones96[:], 1.0)
    epsb = consts.tile([C, 1], F32)
    nc.vector.memset(epsb[:], EPS)

    # pad columns stay zero for the whole kernel
    nc.vector.memset(xact[:, :, 0:1], 0.0)
    nc.vector.memset(xact[:, :, WP - 1:WP], 0.0)

    stats1 = consts.tile([C, NPT, 6], F32)
    stats2 = consts.tile([C, NT, 6], F32)

    # ---------------- phase A: conv1 + GN1 partials ----------------
    with tc.tile_pool(name="psa", bufs=4, space="PSUM") as psa:
        xt = None
        xt_base = 0
        for b0 in range(0, NPT, 2):
            nb = min(2, NPT - b0)
            r0 = 2 * b0                       # first frame row of batch
            if r0 % 8 == 0:
                xt = xin.tile([C, 8, W], F16, tag="xt")
                nrows = min(8, RF - r0)
                nc.sync.dma_start(out=xt[:, 0:nrows, :],
                                  in_=io["xs"][:, r0:r0 + nrows, :])
                xt_base = r0
            pt = psa.tile([M, 2, 512], F32, tag="pta")
            for j in range(nb):
                rr = r0 + 2 * j - xt_base
                nc.tensor.matmul(out=pt[:, j, :], lhsT=w1t[:],
                                 rhs=xt[:, rr:rr + 2, :],
                                 start=True, stop=True)
            nc.scalar.copy(
                out=xact[:, r0:r0 + 2 * nb, 1:W + 1].rearrange(
                    "p (n r) w -> p n r w", r=2),
                in_=pt[0:C, 0:nb, :].rearrange("p n (r w) -> p n r w", w=W))
            for j in range(nb):
                ti = b0 + j
                if ti == 0:
                    src = pt[0:C, j, 256:512]
                elif ti == NPT - 1:
                    src = pt[0:C, j, 0:256]
                else:
                    src = pt[0:C, j, :]
                nc.vector.bn_stats(out=stats1[:, ti, :], in_=src)

        # ---- GN1 partial reduction + AllReduce input
        mv1 = consts.tile([C, 2], F32)
        nc.vector.bn_aggr(out=mv1[:], in_=stats1[:])
        pack1 = consts.tile([C, 2], F32)
        nc.vector.tensor_add(out=pack1[:, 0:1], in0=mv1[:, 0:1],
                             in1=cols[:, 0:1])
        t1sq = consts.tile([C, 1], F32)
        nc.vector.tensor_mul(out=t1sq[:], in0=pack1[:, 0:1],
                             in1=pack1[:, 0:1])
        nc.vector.tensor_add(out=pack1[:, 1:2], in0=mv1[:, 1:2], in1=t1sq[:])
        spt = psa.tile([M, 2, 512], F32, tag="pta")
        nc.tensor.matmul(out=spt[0:1, 0, 0:2], lhsT=ones96[:], rhs=pack1[:],
                         start=True, stop=True)
        ar1_in = consts.tile([1, 2], F32)
        nc.scalar.copy(out=ar1_in[:], in_=spt[0:1, 0, 0:2])
    d1i = dram.tile([1, 2], F32)
    d1o = dram.tile([1, 2], F32)
    nc.sync.dma_start(out=d1i[:], in_=ar1_in[:])
    nc.gpsimd.collective_compute("AllReduce", ALU.add, replica_groups=groups,
                                 ins=[d1i.opt()], outs=[d1o.opt()])
    ar1 = consts.tile([C, 2], F32)
    nc.gpsimd.dma_start(out=ar1[:], in_=_bcast(d1o[:], C))

    # ---- GN1 scalars
    mu1 = consts.tile([C, 1], F32)
    nc.vector.tensor_scalar_mul(out=mu1[:], in0=ar1[:, 0:1], scalar1=inv_n)
    var1 = consts.tile([C, 1], F32)
    nc.vector.tensor_scalar_mul(out=var1[:], in0=ar1[:, 1:2], scalar1=inv_n)
    mu1sq = consts.tile([C, 1], F32)
    nc.vector.tensor_mul(out=mu1sq[:], in0=mu1[:], in1=mu1[:])
    nc.vector.tensor_sub(out=var1[:], in0=var1[:], in1=mu1sq[:])
    std1 = consts.tile([C, 1], F32)
    nc.scalar.activation(out=std1[:], in_=var1[:], func=AF.Sqrt, bias=epsb[:])
    inv1 = consts.tile([C, 1], F32)
    nc.vector.reciprocal(out=inv1[:], in_=std1[:])
    scale1 = consts.tile([C, 1], F32)
    nc.vector.tensor_mul(out=scale1[:], in0=inv1[:], in1=cols[:, 1:2])
    bias1 = consts.tile([C, 1], F32)
    nc.vector.tensor_sub(out=bias1[:], in0=cols[:, 0:1], in1=mu1[:])
    nc.vector.tensor_mul(out=bias1[:], in0=bias1[:], in1=scale1[:])
    nc.vector.tensor_add(out=bias1[:], in0=bias1[:], in1=cols[:, 2:3])

    # ---- GN1 apply + Gelu in place (strided: pad columns untouched)
    rchunk = max(2, (RF + 9) // 10)
    r = 0
    while r < RF:
        rr = min(rchunk, RF - r)
        nc.scalar.activation(out=xact[:, r:r + rr, 1:W + 1],
                             in_=xact[:, r:r + rr, 1:W + 1],
                             func=AF.Gelu, bias=bias1[:], scale=scale1[:])
        r += rr
    nc.vector.tensor_scalar_mul(out=xact[:, 0:1, :], in0=xact[:, 0:1, :],
                                scalar1=em[:, 0:1])
    nc.vector.tensor_scalar_mul(out=xact[:, RF - 1:RF, :],
                                in0=xact[:, RF - 1:RF, :], scalar1=em[:, 1:2])

    # ---------------- phase B: branch convs + Gelu + sum ----------------
    branches = [(0, BR_LR), (0, BR_LDIAG), (3, BR_TD), (3, BR_RDIAG)]
    with tc.tile_pool(name="psb", bufs=2, space="PSUM") as psb:
        for t in range(NT):
            pr = 2 * t + 1
            pt = psb.tile([M, 4, 512], F32, tag="ptb")
            for b, (wsel, ds) in enumerate(branches):
                for j, (dh, dw) in enumerate(ds):
                    bi = wsel + j
                    nc.tensor.matmul(
                        out=pt[:, b, :],
                        lhsT=wbm[:, bi * M:(bi + 1) * M],
                        rhs=xact[0:C, pr + dh:pr + dh + 2,
                                 1 + dw:1 + dw + W],
                        start=(j == 0), stop=(j == 2))
            g = gst.tile([C, 4, 512], F16, tag="g")
            nc.scalar.activation(out=g[:, 0:2, :], in_=pt[0:C, 0:2, :],
                                 func=AF.Gelu, bias=cols[:, 3:4])
            nc.scalar.activation(out=g[:, 2:4, :], in_=pt[0:C, 2:4, :],
                                 func=AF.Gelu, bias=cols[:, 4:5])
            o1 = tmp.tile([C, 512], F16, tag="o1")
            o2 = tmp.tile([C, 512], F16, tag="o2")
            nc.vector.tensor_add(out=o1[:], in0=g[:, 0, :], in1=g[:, 1, :])
            nc.gpsimd.tensor_add(out=o2[:], in0=g[:, 2, :], in1=g[:, 3, :])
            od = opre[:, 512 * t:512 * (t + 1)]
            nc.vector.tensor_add(out=od, in0=o1[:], in1=o2[:])
            nc.vector.bn_stats(out=stats2[:, t, :], in_=od)

        # ---- GN2 partial reduction + AllReduce input
        mv2 = consts.tile([C, 2], F32)
        nc.vector.bn_aggr(out=mv2[:], in_=stats2[:])
        pack2 = consts.tile([C, 2], F32)
        nc.vector.tensor_copy(out=pack2[:, 0:1], in_=mv2[:, 0:1])
        t2sq = consts.tile([C, 1], F32)
        nc.vector.tensor_mul(out=t2sq[:], in0=mv2[:, 0:1], in1=mv2[:, 0:1])
        nc.vector.tensor_add(out=pack2[:, 1:2], in0=mv2[:, 1:2], in1=t2sq[:])
        spt2 = psb.tile([M, 4, 512], F32, tag="ptb")
        nc.tensor.matmul(out=spt2[0:1, 0, 0:2], lhsT=ones96[:], rhs=pack2[:],
                         start=True, stop=True)
        ar2_in = consts.tile([1, 2], F32)
        nc.scalar.copy(out=ar2_in[:], in_=spt2[0:1, 0, 0:2])
    d2i = dram.tile([1, 2], F32)
    d2o = dram.tile([1, 2], F32)
    nc.sync.dma_start(out=d2i[:], in_=ar2_in[:])
    nc.gpsimd.collective_compute("AllReduce", ALU.add, replica_groups=groups,
                                 ins=[d2i.opt()], outs=[d2o.opt()])
    ar2 = consts.tile([C, 2], F32)
    nc.gpsimd.dma_start(out=ar2[:], in_=_bcast(d2o[:], C))

    # ---- GN2 scalars; fold gamma2/std into conv3, mean into bias
    mu2 = consts.tile([C, 1], F32)
    nc.vector.tensor_scalar_mul(out=mu2[:], in0=ar2[:, 0:1], scalar1=inv_n)
    var2 = consts.tile([C, 1], F32)
    nc.vector.tensor_scalar_mul(out=var2[:], in0=ar2[:, 1:2], scalar1=inv_n)
    mu2sq = consts.tile([C, 1], F32)
    nc.vector.tensor_mul(out=mu2sq[:], in0=mu2[:], in1=mu2[:])
    nc.vector.tensor_sub(out=var2[:], in0=var2[:], in1=mu2sq[:])
    std2 = consts.tile([C, 1], F32)
    nc.scalar.activation(out=std2[:], in_=var2[:], func=AF.Sqrt, bias=epsb[:])
    inv2 = consts.tile([C, 1], F32)
    nc.vector.reciprocal(out=inv2[:], in_=std2[:])
    w3ts = consts.tile([C, M], F16)
    nc.vector.tensor_scalar_mul(out=w3ts[:], in0=w3gt[:], scalar1=inv2[:])
    s2 = consts.tile([C, 1], F32)
    nc.vector.tensor_mul(out=s2[:], in0=inv2[:], in1=mu2[:])
    ccol = consts.tile([C, 1], F32)
    nc.vector.tensor_mul(out=ccol[:], in0=s2[:], in1=cols[:, 6:7])
    nc.vector.tensor_sub(out=ccol[:], in0=cols[:, 5:6], in1=ccol[:])

    # ---------------- phase C: conv3 + bias, DMA out ----------------
    with tc.tile_pool(name="psc", bufs=4, space="PSUM") as psc:
        for b0 in range(0, NT, 2):
            nb = min(2, NT - b0)
            pc = psc.tile([M, 2, 512], F32, tag="ptc")
            for j in range(nb):
                tt = b0 + j
                nc.tensor.matmul(out=pc[:, j, :], lhsT=w3ts[:],
                                 rhs=opre[:, 512 * tt:512 * (tt + 1)],
                                 start=True, stop=True)
            o = ost.tile([C, 2, 512], F32, tag="o")
            nc.vector.tensor_scalar(out=o[:, 0:nb, :], in0=pc[0:C, 0:nb, :],
                                    scalar1=ccol[:], scalar2=None,
                                    op0=ALU.add)
            nc.sync.dma_start(
                out=io["out"][:, 2 * b0:2 * b0 + 2 * nb, :].rearrange(
                    "p (n r) w -> p n r w", r=2),
                in_=o[:, 0:nb, :].rearrange("p n (r w) -> p n r w", w=W))


def build_program(rows_half=ROWS_HALF, n_cores=N_CORES, groups=None):
    import contextlib
    if groups is None:
        groups = [[i, i + 1] for i in range(0, n_cores, 2)]
    RF = rows_half + 2
    nc = bacc.Bacc("TRN2", target_bir_lowering=False, debug=False,
                   enable_asserts=False, num_devices=n_cores)
    io = {
        "xs": nc.dram_tensor("xs", [C, RF, W], F16, kind="ExternalInput").ap(),
        "em": nc.dram_tensor("em", [1, 2], F32, kind="ExternalInput").ap(),
        "w1t": nc.dram_tensor("w1t", [C, M], F16, kind="ExternalInput").ap(),
        "wbm": nc.dram_tensor("wbm", [C, 6 * M], F16,
                              kind="ExternalInput").ap(),
        "w3gt": nc.dram_tensor("w3gt", [C, M], F32, kind="ExternalInput").ap(),
        "cols": nc.dram_tensor("cols", [C, 7], F32, kind="ExternalInput").ap(),
        "out": nc.dram_tensor("out", [C, rows_half, W], F32,
                              kind="ExternalOutput").ap(),
    }
    with tile.TileContext(nc) as tc:
        with contextlib.ExitStack() as ctx:
            _emit(nc, tc, ctx, rows_half, groups, io)
    nc.compile()
    return nc


def host_inputs(x, w1, b1, w21, b21, w22, b22, w3, b3,
                gn1_w, gn1_b, gn2_w, gn2_b, rows_half=ROWS_HALF):
    x = np.asarray(x, np.float32)
    nb_, _, hh, _ = x.shape
    halves = hh // rows_half
    n_cores = nb_ * halves
    w1 = np.asarray(w1, np.float32)
    w21 = np.asarray(w21, np.float32)
    w22 = np.asarray(w22, np.float32)
    w3 = np.asarray(w3, np.float32)

    w1t = np.zeros((C, M), np.float16)
    w1t[:, 0:C] = w1.T
    wbm = np.zeros((C, 6 * M), np.float16)
    for wi, wmat in enumerate((w21, w22)):
        wt = np.ascontiguousarray(wmat.T).astype(np.float16)
        for j in range(3):
            blk = np.zeros((C, M), np.float16)
            blk[32 * j:32 * j + 32, 0:C] = wt[32 * j:32 * j + 32, :]
            wbm[:, (3 * wi + j) * M:(3 * wi + j + 1) * M] = blk
    w3gt = np.zeros((C, M), np.float32)
    w3gt[:, 0:C] = (w3 * np.asarray(gn2_w)[None, :]).T
    shared = {
        "w1t": w1t,
        "wbm": wbm,
        "w3gt": w3gt,
        "cols": np.ascontiguousarray(np.stack(
            [np.asarray(b1, np.float32), np.asarray(gn1_w, np.float32),
             np.asarray(gn1_b, np.float32), np.asarray(b21, np.float32),
             np.asarray(b22, np.float32),
             (np.asarray(b3) + w3 @ np.asarray(gn2_b)).astype(np.float32),
             (w3 * np.asarray(gn2_w)[None, :]).sum(1).astype(np.float32)],
            axis=1)),
    }
    in_maps = []
    for k in range(n_cores):
        bidx, half = k // halves, k % halves
        h0 = half * rows_half
        slab = np.zeros((C, rows_half + 2, W), np.float16)
        lo, hi = h0 - 1, h0 + rows_half + 1
        slo, shi = max(lo, 0), min(hi, hh)
        slab[:, slo - lo:slo - lo + (shi - slo), :] = \
            x[bidx, :, slo:shi, :].astype(np.float16)
        em = np.array([[1.0 if lo >= 0 else 0.0,
                        1.0 if hi <= hh else 0.0]], np.float32)
        in_maps.append({"xs": slab, "em": em, **shared})
    return in_maps


_PROGRAM = None


def kernel(x, w1, b1, w21, b21, w22, b22, w3, b3, gn1_w, gn1_b, gn2_w, gn2_b):
    global _PROGRAM
    from concourse.bass_utils import run_bass_kernel_spmd
    from concourse.bass_interp import get_hw_module
    if _PROGRAM is None:
        nc = build_program()
        nc.m = get_hw_module(nc.m)
        _PROGRAM = nc
    nc = _PROGRAM
    in_maps = host_inputs(x, w1, b1, w21, b21, w22, b22, w3, b3,
                          gn1_w, gn1_b, gn2_w, gn2_b)
    res = run_bass_kernel_spmd(nc, in_maps, core_ids=list(range(N_CORES)))
    out = np.empty((B, C, H, W), np.float32)
    for k in range(N_CORES):
        bidx, half = k // 2, k % 2
        out[bidx, :, half * ROWS_HALF:(half + 1) * ROWS_HALF, :] = \
            res.results[k]["out"]
    return out


# revision 8
# speedup vs baseline: 1.2288x; 1.0032x over previous
"""Trainium2 Bass kernel for the AxialShift block (4x96x256x256, fp32).

Self-contained: builds an 8-core SPMD Bass program (one NeuronCore per
half-sample), compiles it once, and runs it via run_bass_kernel_spmd.

Sharding: core k handles sample k//2, horizontal half k%2 (128 rows).
Per-core pipeline (x read once as fp16, out written once):
  phase A : conv1 (fp16 matmul, M padded to 128 for fast weight load)
            over the 130-row frame (128 true rows + 1 halo row each
            side); stored fp16 into a zero-padded [96, 130, 258] frame;
            per-channel bn_stats partials on the true region.
  AR1     : 8-byte AllReduce of GroupNorm-1 partials over the 2-core
            group sharing each sample.
  GN-apply: fused per-channel scale/bias + erf-Gelu in place (strided,
            pad columns stay zero).
  phase B : the 4 axial-shift branch convs as 12 chunk-masked K=96
            matmuls per 512-pixel tile (shifts are free-dim offsets
            into the padded frame), fused bias+Gelu on ScalarE, branch
            sum split across VectorE/GpSimd, bn_stats partials for GN2.
  AR2     : second 8-byte AllReduce.
  phase C : GroupNorm-2 folded into conv3 (weights scaled by gamma2/std
            on device; mean/bias folded into a per-channel bias applied
            by the VectorE PSUM->SBUF evacuation), DMA out.
"""
import sys

sys.path.insert(0, "/opt/trn_rl_repo")

import numpy as np

import concourse.bass as bass
import concourse.bacc as bacc
import concourse.tile as tile
from concourse import mybir

F32 = mybir.dt.float32
F16 = mybir.dt.float16

C = 96
M = 128           # matmul output width (96 channels + 32 zero pad, FWL)
H = 256
W = 256
B = 4
WP = W + 2
N_CORES = 8
ROWS_HALF = H // 2
EPS = 1e-5
AF = mybir.ActivationFunctionType
ALU = mybir.AluOpType

# (dh, dw) read offsets per chunk j=0,1,2 (s_j = -1, 0, +1):
BR_LR = [(0, 1), (0, 0), (0, -1)]
BR_LDIAG = [(1, 1), (0, 0), (-1, -1)]
BR_TD = [(1, 0), (0, 0), (-1, 0)]
BR_RDIAG = [(1, -1), (0, 0), (-1, 1)]


def _bcast(ap, nparts):
    return bass.AP(tensor=ap.tensor, offset=ap.offset,
                   ap=[[0, nparts]] + list(ap.ap[1:]))


def _emit(nc, tc, ctx, RH, groups, io):
    RF = RH + 2
    NT = RH // 2
    NPT = RF // 2
    inv_n = 1.0 / (len(groups[0]) * C)

    consts = ctx.enter_context(tc.tile_pool(name="consts", bufs=1))
    big = ctx.enter_context(tc.tile_pool(name="big", bufs=1))
    xin = ctx.enter_context(tc.tile_pool(name="xin", bufs=3))
    gst = ctx.enter_context(tc.tile_pool(name="gst", bufs=3))
    tmp = ctx.enter_context(tc.tile_pool(name="tmp", bufs=4))
    ost = ctx.enter_context(tc.tile_pool(name="ost", bufs=4))
    dram = ctx.enter_context(tc.tile_pool(name="dram", bufs=1, space="DRAM"))

    # ---------------- resident buffers & constants ----------------
    xact = big.tile([C, RF, WP], F16)
    opre = big.tile([C, RH * W], F16)

    w1t = consts.tile([C, M], F16)
    nc.sync.dma_start(out=w1t[:], in_=io["w1t"][:])
    wbm = consts.tile([C, 6 * M], F16)
    nc.sync.dma_start(out=wbm[:], in_=io["wbm"][:])
    w3gt = consts.tile([C, M], F32)
    nc.sync.dma_start(out=w3gt[:], in_=io["w3gt"][:])
    cols = consts.tile([C, 7], F32)
    nc.sync.dma_start(out=cols[:], in_=io["cols"][:])
    em = consts.tile([C, 2], F32)
    nc.gpsimd.dma_start(out=em[:], in_=_bcast(io["em"][:], C))
    ones96 = consts.tile([C, 1], F32)
    nc.vector.memset(ones96[:], 1.0)
    epsb = consts.tile([C, 1], F32)
    nc.vector.memset(epsb[:], EPS)

    # pad columns stay zero for the whole kernel
    nc.vector.memset(xact[:, :, 0:1], 0.0)
    nc.vector.memset(xact[:, :, WP - 1:WP], 0.0)

    stats1 = consts.tile([C, NPT, 6], F32)
    stats2 = consts.tile([C, NT, 6], F32)

    # ---------------- phase A: conv1 + GN1 partials ----------------
    with tc.tile_pool(name="psa", bufs=4, space="PSUM") as psa:
        xt = None
        xt_base = 0
        for b0 in range(0, NPT, 2):
            nb = min(2, NPT - b0)
            r0 = 2 * b0                       # first frame row of batch
            if r0 % 8 == 0:
                xt = xin.tile([C, 8, W], F16, tag="xt")
                nrows = min(8, RF - r0)
                nc.sync.dma_start(out=xt[:, 0:nrows, :],
                                  in_=io["xs"][:, r0:r0 + nrows, :])
                xt_base = r0
            pt = psa.tile([M, 2, 512], F32, tag="pta")
            for j in range(nb):
                rr = r0 + 2 * j - xt_base
                nc.tensor.matmul(out=pt[:, j, :], lhsT=w1t[:],
                                 rhs=xt[:, rr:rr + 2, :],
                                 start=True, stop=True)
            nc.scalar.copy(
                out=xact[:, r0:r0 + 2 * nb, 1:W + 1].rearrange(
                    "p (n r) w -> p n r w", r=2),
                in_=pt[0:C, 0:nb, :].rearrange("p n (r w) -> p n r w", w=W))
            for j in range(nb):
                ti = b0 + j
                if ti == 0:
                    src = pt[0:C, j, 256:512]
                elif ti == NPT - 1:
                    src = pt[0:C, j, 0:256]
                else:
                    src = pt[0:C, j, :]
                nc.vector.bn_stats(out=stats1[:, ti, :], in_=src)

        # ---- GN1 partial reduction + AllReduce input
        mv1 = consts.tile([C, 2], F32)
        nc.vector.bn_aggr(out=mv1[:], in_=stats1[:])
        pack1 = consts.tile([C, 2], F32)
        nc.vector.tensor_add(out=pack1[:, 0:1], in0=mv1[:, 0:1],
                             in1=cols[:, 0:1])
        t1sq = consts.tile([C, 1], F32)
        nc.vector.tensor_mul(out=t1sq[:], in0=pack1[:, 0:1],
                             in1=pack1[:, 0:1])
        nc.vector.tensor_add(out=pack1[:, 1:2], in0=mv1[:, 1:2], in1=t1sq[:])
        spt = psa.tile([M, 2, 512], F32, tag="pta")
        nc.tensor.matmul(out=spt[0:1, 0, 0:2], lhsT=ones96[:], rhs=pack1[:],
                         start=True, stop=True)
        ar1_in = consts.tile([1, 2], F32)
        nc.scalar.copy(out=ar1_in[:], in_=spt[0:1, 0, 0:2])
    d1i = dram.tile([1, 2], F32)
    d1o = dram.tile([1, 2], F32)
    nc.sync.dma_start(out=d1i[:], in_=ar1_in[:])
    nc.gpsimd.collective_compute("AllReduce", ALU.add, replica_groups=groups,
                                 ins=[d1i.opt()], outs=[d1o.opt()])
    ar1 = consts.tile([C, 2], F32)
    nc.gpsimd.dma_start(out=ar1[:], in_=_bcast(d1o[:], C))

    # ---- GN1 scalars
    mu1 = consts.tile([C, 1], F32)
    nc.vector.tensor_scalar_mul(out=mu1[:], in0=ar1[:, 0:1], scalar1=inv_n)
    var1 = consts.tile([C, 1], F32)
    nc.vector.tensor_scalar_mul(out=var1[:], in0=ar1[:, 1:2], scalar1=inv_n)
    mu1sq = consts.tile([C, 1], F32)
    nc.vector.tensor_mul(out=mu1sq[:], in0=mu1[:], in1=mu1[:])
    nc.vector.tensor_sub(out=var1[:], in0=var1[:], in1=mu1sq[:])
    std1 = consts.tile([C, 1], F32)
    nc.scalar.activation(out=std1[:], in_=var1[:], func=AF.Sqrt, bias=epsb[:])
    inv1 = consts.tile([C, 1], F32)
    nc.vector.reciprocal(out=inv1[:], in_=std1[:])
    scale1 = consts.tile([C, 1], F32)
    nc.vector.tensor_mul(out=scale1[:], in0=inv1[:], in1=cols[:, 1:2])
    bias1 = consts.tile([C, 1], F32)
    nc.vector.tensor_sub(out=bias1[:], in0=cols[:, 0:1], in1=mu1[:])
    nc.vector.tensor_mul(out=bias1[:], in0=bias1[:], in1=scale1[:])
    nc.vector.tensor_add(out=bias1[:], in0=bias1[:], in1=cols[:, 2:3])

    # ---- GN1 apply + Gelu in place (strided: pad columns untouched)
    rchunk = max(2, (RF + 9) // 10)
    r = 0
    while r < RF:
        rr = min(rchunk, RF - r)
        nc.scalar.activation(out=xact[:, r:r + rr, 1:W + 1],
                             in_=xact[:, r:r + rr, 1:W + 1],
                             func=AF.Gelu, bias=bias1[:], scale=scale1[:])
        r += rr
    nc.vector.tensor_scalar_mul(out=xact[:, 0:1, :], in0=xact[:, 0:1, :],
                                scalar1=em[:, 0:1])
    nc.vector.tensor_scalar_mul(out=xact[:, RF - 1:RF, :],
                                in0=xact[:, RF - 1:RF, :], scalar1=em[:, 1:2])

    # ---------------- phase B: branch convs + Gelu + sum ----------------
    branches = [(0, BR_LR), (0, BR_LDIAG), (3, BR_TD), (3, BR_RDIAG)]
    with tc.tile_pool(name="psb", bufs=2, space="PSUM") as psb:
        for t in range(NT):
            pr = 2 * t + 1
            pt = psb.tile([M, 4, 512], F32, tag="ptb")
            for b, (wsel, ds) in enumerate(branches):
                for j, (dh, dw) in enumerate(ds):
                    bi = wsel + j
                    nc.tensor.matmul(
                        out=pt[:, b, :],
                        lhsT=wbm[:, bi * M:(bi + 1) * M],
                        rhs=xact[0:C, pr + dh:pr + dh + 2,
                                 1 + dw:1 + dw + W],
                        start=(j == 0), stop=(j == 2))
            g = gst.tile([C, 4, 512], F16, tag="g")
            nc.scalar.activation(out=g[:, 0:2, :], in_=pt[0:C, 0:2, :],
                                 func=AF.Gelu, bias=cols[:, 3:4])
            nc.scalar.activation(out=g[:, 2:4, :], in_=pt[0:C, 2:4, :],
                                 func=AF.Gelu, bias=cols[:, 4:5])
            o1 = tmp.tile([C, 512], F16, tag="o1")
            o2 = tmp.tile([C, 512], F16, tag="o2")
            nc.vector.tensor_add(out=o1[:], in0=g[:, 0, :], in1=g[:, 1, :])
            nc.vector.tensor_add(out=o2[:], in0=g[:, 2, :], in1=g[:, 3, :])
            od = opre[:, 512 * t:512 * (t + 1)]
            nc.vector.tensor_add(out=od, in0=o1[:], in1=o2[:])
            nc.vector.bn_stats(out=stats2[:, t, :], in_=od)

        # ---- GN2 partial reduction + AllReduce input
        mv2 = consts.tile([C, 2], F32)
        nc.vector.bn_aggr(out=mv2[:], in_=stats2[:])
        pack2 = consts.tile([C, 2], F32)
        nc.vector.tensor_copy(out=pack2[:, 0:1], in_=mv2[:, 0:1])
        t2sq = consts.tile([C, 1], F32)
        nc.vector.tensor_mul(out=t2sq[:], in0=mv2[:, 0:1], in1=mv2[:, 0:1])
        nc.vector.tensor_add(out=pack2[:, 1:2], in0=mv2[:, 1:2], in1=t2sq[:])
        spt2 = psb.tile([M, 4, 512], F32, tag="ptb")
        nc.tensor.matmul(out=spt2[0:1, 0, 0:2], lhsT=ones96[:], rhs=pack2[:],
                         start=True, stop=True)
        ar2_in = consts.tile([1, 2], F32)
        nc.scalar.copy(out=ar2_in[:], in_=spt2[0:1, 0, 0:2])
    d2i = dram.tile([1, 2], F32)
    d2o = dram.tile([1, 2], F32)
    nc.sync.dma_start(out=d2i[:], in_=ar2_in[:])
    nc.gpsimd.collective_compute("AllReduce", ALU.add, replica_groups=groups,
                                 ins=[d2i.opt()], outs=[d2o.opt()])
    ar2 = consts.tile([C, 2], F32)
    nc.gpsimd.dma_start(out=ar2[:], in_=_bcast(d2o[:], C))

    # ---- GN2 scalars; fold gamma2/std into conv3, mean into bias
    mu2 = consts.tile([C, 1], F32)
    nc.vector.tensor_scalar_mul(out=mu2[:], in0=ar2[:, 0:1], scalar1=inv_n)
    var2 = consts.tile([C, 1], F32)
    nc.vector.tensor_scalar_mul(out=var2[:], in0=ar2[:, 1:2], scalar1=inv_n)
    mu2sq = consts.tile([C, 1], F32)
    nc.vector.tensor_mul(out=mu2sq[:], in0=mu2[:], in1=mu2[:])
    nc.vector.tensor_sub(out=var2[:], in0=var2[:], in1=mu2sq[:])
    std2 = consts.tile([C, 1], F32)
    nc.scalar.activation(out=std2[:], in_=var2[:], func=AF.Sqrt, bias=epsb[:])
    inv2 = consts.tile([C, 1], F32)
    nc.vector.reciprocal(out=inv2[:], in_=std2[:])
    w3ts = consts.tile([C, M], F16)
    nc.vector.tensor_scalar_mul(out=w3ts[:], in0=w3gt[:], scalar1=inv2[:])
    s2 = consts.tile([C, 1], F32)
    nc.vector.tensor_mul(out=s2[:], in0=inv2[:], in1=mu2[:])
    ccol = consts.tile([C, 1], F32)
    nc.vector.tensor_mul(out=ccol[:], in0=s2[:], in1=cols[:, 6:7])
    nc.vector.tensor_sub(out=ccol[:], in0=cols[:, 5:6], in1=ccol[:])

    # ---------------- phase C: conv3 + bias, DMA out ----------------
    with tc.tile_pool(name="psc", bufs=4, space="PSUM") as psc:
        for b0 in range(0, NT, 2):
            nb = min(2, NT - b0)
            pc = psc.tile([M, 2, 512], F32, tag="ptc")
            for j in range(nb):
                tt = b0 + j
                nc.tensor.matmul(out=pc[:, j, :], lhsT=w3ts[:],
                                 rhs=opre[:, 512 * tt:512 * (tt + 1)],
                                 start=True, stop=True)
            o = ost.tile([C, 2, 512], F32, tag="o")
            nc.vector.tensor_scalar(out=o[:, 0:nb, :], in0=pc[0:C, 0:nb, :],
                                    scalar1=ccol[:], scalar2=None,
                                    op0=ALU.add)
            nc.sync.dma_start(
                out=io["out"][:, 2 * b0:2 * b0 + 2 * nb, :].rearrange(
                    "p (n r) w -> p n r w", r=2),
                in_=o[:, 0:nb, :].rearrange("p n (r w) -> p n r w", w=W))


def build_program(rows_half=ROWS_HALF, n_cores=N_CORES, groups=None):
    import contextlib
    if groups is None:
        groups = [[i, i + 1] for i in range(0, n_cores, 2)]
    RF = rows_half + 2
    nc = bacc.Bacc("TRN2", target_bir_lowering=False, debug=False,
                   enable_asserts=False, num_devices=n_cores)
    io = {
        "xs": nc.dram_tensor("xs", [C, RF, W], F16, kind="ExternalInput").ap(),
        "em": nc.dram_tensor("em", [1, 2], F32, kind="ExternalInput").ap(),
        "w1t": nc.dram_tensor("w1t", [C, M], F16, kind="ExternalInput").ap(),
        "wbm": nc.dram_tensor("wbm", [C, 6 * M], F16,
                              kind="ExternalInput").ap(),
        "w3gt": nc.dram_tensor("w3gt", [C, M], F32, kind="ExternalInput").ap(),
        "cols": nc.dram_tensor("cols", [C, 7], F32, kind="ExternalInput").ap(),
        "out": nc.dram_tensor("out", [C, rows_half, W], F32,
                              kind="ExternalOutput").ap(),
    }
    with tile.TileContext(nc) as tc:
        with contextlib.ExitStack() as ctx:
            _emit(nc, tc, ctx, rows_half, groups, io)
    nc.compile()
    return nc


def host_inputs(x, w1, b1, w21, b21, w22, b22, w3, b3,
                gn1_w, gn1_b, gn2_w, gn2_b, rows_half=ROWS_HALF):
    x = np.asarray(x, np.float32)
    nb_, _, hh, _ = x.shape
    halves = hh // rows_half
    n_cores = nb_ * halves
    w1 = np.asarray(w1, np.float32)
    w21 = np.asarray(w21, np.float32)
    w22 = np.asarray(w22, np.float32)
    w3 = np.asarray(w3, np.float32)

    w1t = np.zeros((C, M), np.float16)
    w1t[:, 0:C] = w1.T
    wbm = np.zeros((C, 6 * M), np.float16)
    for wi, wmat in enumerate((w21, w22)):
        wt = np.ascontiguousarray(wmat.T).astype(np.float16)
        for j in range(3):
            blk = np.zeros((C, M), np.float16)
            blk[32 * j:32 * j + 32, 0:C] = wt[32 * j:32 * j + 32, :]
            wbm[:, (3 * wi + j) * M:(3 * wi + j + 1) * M] = blk
    w3gt = np.zeros((C, M), np.float32)
    w3gt[:, 0:C] = (w3 * np.asarray(gn2_w)[None, :]).T
    shared = {
        "w1t": w1t,
        "wbm": wbm,
        "w3gt": w3gt,
        "cols": np.ascontiguousarray(np.stack(
            [np.asarray(b1, np.float32), np.asarray(gn1_w, np.float32),
             np.asarray(gn1_b, np.float32), np.asarray(b21, np.float32),
             np.asarray(b22, np.float32),
             (np.asarray(b3) + w3 @ np.asarray(gn2_b)).astype(np.float32),
             (w3 * np.asarray(gn2_w)[None, :]).sum(1).astype(np.float32)],
            axis=1)),
    }
    in_maps = []
    for k in range(n_cores):
        bidx, half = k // halves, k % halves
        h0 = half * rows_half
        slab = np.zeros((C, rows_half + 2, W), np.float16)
        lo, hi = h0 - 1, h0 + rows_half + 1
        slo, shi = max(lo, 0), min(hi, hh)
        slab[:, slo - lo:slo - lo + (shi - slo), :] = \
            x[bidx, :, slo:shi, :].astype(np.float16)
        em = np.array([[1.0 if lo >= 0 else 0.0,
                        1.0 if hi <= hh else 0.0]], np.float32)
        in_maps.append({"xs": slab, "em": em, **shared})
    return in_maps


_PROGRAM = None


def kernel(x, w1, b1, w21, b21, w22, b22, w3, b3, gn1_w, gn1_b, gn2_w, gn2_b):
    global _PROGRAM
    from concourse.bass_utils import run_bass_kernel_spmd
    from concourse.bass_interp import get_hw_module
    if _PROGRAM is None:
        nc = build_program()
        nc.m = get_hw_module(nc.m)
        _PROGRAM = nc
    nc = _PROGRAM
    in_maps = host_inputs(x, w1, b1, w21, b21, w22, b22, w3, b3,
                          gn1_w, gn1_b, gn2_w, gn2_b)
    res = run_bass_kernel_spmd(nc, in_maps, core_ids=list(range(N_CORES)))
    out = np.empty((B, C, H, W), np.float32)
    for k in range(N_CORES):
        bidx, half = k // 2, k % 2
        out[bidx, :, half * ROWS_HALF:(half + 1) * ROWS_HALF, :] = \
            res.results[k]["out"]
    return out
